# revision 1
# baseline (speedup 1.0000x reference)
"""Additive-attention fused kernel for one TRN2 chip (8 NeuronCores).

Math (per batch b):
    q = queries @ W_q.T                       [Q, H]
    k = keys    @ W_k.T                       [K, H]
    scores[q,k] = sum_h w_v[h] * tanh(q[q,h] + k[k,h])
    attn = masked_softmax(scores, valid_len)  (mask: k >= L -> weight 0)
    out  = attn @ values                      [Q, V]

Sharding: data-parallel over batch B=4 x 2-way split of Q -> 8 cores,
each core handles [QH=512, :] of one batch.  No collectives needed.

Key algorithmic trick (v2): tanh(x+y) is a smooth 2-variable kernel, so
it has a rapidly-converging separable expansion.  We use an odd Fourier
series fitted offline (weighted LSQ, frequencies m*pi/L, m=1..M):

    tanh(z) ~ sum_m c_m sin(pi m z / L)            |z| <= 2*BCLAMP
    sin(w(x+y)) = sin(wx)cos(wy) + cos(wx)sin(wy)  -> rank 2 per freq

so scores collapse to ONE matmul with contraction dim H*2M = 2048:
    scores[k,q] = sum_{h,m} [c_m w_v[h] sin_m(q_h)] cos_m(k_h)
                           + [c_m w_v[h] cos_m(q_h)] sin_m(k_h)
This replaces 268M ACT tanh evaluations (~230 us on ScalarE, the v1
bottleneck) with ~100K ACT sin evaluations + PE matmuls.

Device pipeline per core (host pre-transposes Q/K/W to [D, n] fp16 and
pre-multiplies the valid-length mask into vaug = [values | ones] fp16 —
pure layout/sharding prep; all real compute stays on-chip):
  - Project with W on PE (fp16), clamp to +-BCLAMP -> qT2 [128, 512]
    (rows 0:64 = q_proj.T h-major, rows 64:128 duplicate), kT2 [128,1024].
  - Per frequency m (outer loop): DVE fp16 chain t = nu_m*x + phase
    (phase 0 / 0.25 for the sin/cos halves of the 128 partitions),
    f = t - round(t) via the fp32-magic add (the fp32 ALU rounds), one
    fused ACT Sin over [q-feat | k-feat] -> fp16; q features scaled by
    w_v[h]*c_m (per-partition scalar, sign flips cancel q vs k).
    PE immediately accumulates the 8 scoresT banks [k, q] per m.
  - Two wide exp calls PSUM->SBUF fp16 (no max-subtraction needed:
    |scores| <= sum|w_v| ~ 6.4, and masked weights become 0 through the
    masked vaug, matching the reference's exactly-0 masked softmax).
  - attn @ [values | ones]: accumulated matmul over k-blocks gives the
    output numerator and the softmax denominator; epilogue divides.
PSUM is one tag-shared pool: projections -> 8 score banks -> 4 attn@V
accumulators reuse the same 16 KB with Tile-inserted dependencies.
"""

import math

import numpy as np

B, QFULL, KK = 4, 1024, 1024
D, H, V = 256, 64, 256
QH = 512            # Q rows per core
NCORES = 8
NB = KK // 128      # 8 k-blocks

# tanh(z) ~ sum_m C[m-1] * sin(pi*m*z / FL): fitted below
FM = 10             # number of frequencies
FL = 10.5           # half-period
BCLAMP = 5.5        # clamp q/k projections to +-BCLAMP (|z| <= 11)

_STATE = {}


def _fit_coeffs():
    z = np.linspace(-2 * BCLAMP, 2 * BCLAMP, 4001)
    w = np.exp(-z ** 2 / (2 * 2.2)) + 1e-4
    A = np.sin(np.pi / FL * np.outer(z, np.arange(1, FM + 1)))
    c = np.linalg.lstsq(A * w[:, None] ** 0.5, np.tanh(z) * w ** 0.5, rcond=None)[0]
    return c.astype(np.float32)


COEFFS = _fit_coeffs()


def _build_nc(n_iters=1):
    import contextlib
    import concourse.tile as tile
    from concourse import bacc, mybir

    F32 = mybir.dt.float32
    F16 = mybir.dt.float16
    Sin = mybir.ActivationFunctionType.Sin
    Exp = mybir.ActivationFunctionType.Exp
    AOp = mybir.AluOpType
    TWO_PI = 2.0 * math.pi

    nc = bacc.Bacc()
    qT_d = nc.declare_dram_parameter("queriesT", [D, QH], F16, isOutput=False)
    kT_d = nc.declare_dram_parameter("keysT", [D, KK], F16, isOutput=False)
    va_d = nc.declare_dram_parameter("vaug", [KK, V + 1], F16, isOutput=False)
    wqT_d = nc.declare_dram_parameter("wqT", [D, H], F16, isOutput=False)
    wkT_d = nc.declare_dram_parameter("wkT", [D, H], F16, isOutput=False)
    wvc_d = nc.declare_dram_parameter("wvc", [128, FM], F32, isOutput=False)
    out_d = nc.declare_dram_parameter("out", [QH, V], F32, isOutput=True)

    with tile.TileContext(nc) as tc:
        with (
            tc.tile_pool(name="singles", bufs=1) as singles,
            tc.tile_pool(name="temps", bufs=4) as temps,
            tc.tile_pool(name="outp", bufs=2) as outp,
            # one PSUM pool, tag-shared slot = all 8 banks; successive
            # allocations (projections -> scores -> attn@V) reuse the space
            # with Tile-inserted dependencies.
            tc.tile_pool(name="ps_big", bufs=1, space="PSUM") as ps_big,
            # n_iters > 1: loop the whole body for wall-clock timing
            # (axon dispatch latency is ~40 ms; amortize to resolve ~us).
            tc.For_i(0, n_iters, 1,
                     hint_engines=(mybir.EngineType.PE, mybir.EngineType.DVE,
                                   mybir.EngineType.Activation,
                                   mybir.EngineType.SP, mybir.EngineType.Pool),
                     staggered_reset=True)
            if n_iters > 1 else contextlib.nullcontext(),
        ):
            # -------- stage inputs (already transposed on host) --------
            # DMA order: small weights first so projections start early;
            # values last (only needed in the tail).
            wqT, wkT = [], []
            for c in range(2):
                t = singles.tile([128, H], F16, tag=f"wqT{c}", name=f"wqT{c}")
                nc.sync.dma_start(t, wqT_d[c * 128:(c + 1) * 128, :])
                wqT.append(t)
                t2 = singles.tile([128, H], F16, tag=f"wkT{c}", name=f"wkT{c}")
                nc.sync.dma_start(t2, wkT_d[c * 128:(c + 1) * 128, :])
                wkT.append(t2)
            wvc_sb = singles.tile([128, FM], F32, tag="wvc", name="wvc")
            nc.sync.dma_start(wvc_sb, wvc_d[:, :])
            queriesT = []
            for c in range(2):
                t = singles.tile([128, QH], F16, tag=f"qTc{c}", name=f"qTc{c}")
                nc.sync.dma_start(t, qT_d[c * 128:(c + 1) * 128, :])
                queriesT.append(t)
            keysT = []
            for c in range(2):
                t = singles.tile([128, KK], F16, tag=f"kTc{c}", name=f"kTc{c}")
                nc.sync.dma_start(t, kT_d[c * 128:(c + 1) * 128, :])
                keysT.append(t)
            v_aug = []
            for i in range(KK // 128):
                va = singles.tile([128, V + 1], F16, tag=f"vaug{i}", name=f"vaug{i}")
                nc.sync.dma_start(va, va_d[i * 128:(i + 1) * 128, :])
                v_aug.append(va)

            # per-partition phase offsets: q side [sin; cos], k side [cos; sin]
            # (cos(u) = sin(u + pi/2) -> +0.25 period offset)
            MAGIC = float(1.5 * 2 ** 23)
            bq = singles.tile([128, 1], F32, tag="bq", name="bq")
            nc.vector.memset(bq[0:H, :], 0.0)
            nc.vector.memset(bq[H:128, :], 0.25)
            bk = singles.tile([128, 1], F32, tag="bk", name="bk")
            nc.vector.memset(bk[0:H, :], 0.25)
            nc.vector.memset(bk[H:128, :], 0.0)

            # ------------- projections (+clamp -> fp16), h-duplicated -------------
            # q and k projections into one [64, 3*512] psum allocation so the
            # big-pool slot is claimed once for the whole prologue.
            qT2 = singles.tile([128, QH], F16, tag="qT2", name="qT2")
            kT2 = singles.tile([128, KK], F16, tag="kT2", name="kT2")
            psp = ps_big.tile([H, 3, 512], F32, tag="big", name="psp")
            for c in range(2):
                nc.tensor.matmul(psp[:, 0, :], wqT[c], queriesT[c], start=(c == 0), stop=(c == 1))
            nc.vector.tensor_scalar(qT2[0:H, :], psp[:, 0, :], BCLAMP, -BCLAMP,
                                    AOp.min, AOp.max)
            nc.sync.dma_start(qT2[H:128, :], qT2[0:H, :])
            for kc in range(2):
                for c in range(2):
                    nc.tensor.matmul(psp[:, 1 + kc, :], wkT[c],
                                     keysT[c][:, kc * 512:(kc + 1) * 512],
                                     start=(c == 0), stop=(c == 1))
                nc.vector.tensor_scalar(kT2[0:H, kc * 512:(kc + 1) * 512], psp[:, 1 + kc, :],
                                        BCLAMP, -BCLAMP, AOp.min, AOp.max)
            nc.sync.dma_start(kT2[H:128, :], kT2[0:H, :])

            # ------- sin/cos features per frequency + scoresT accumulation -------
            # All-fp16 chain (DVE ALU is fp32 internally, I/O fp16 runs the
            # fast 4x/2x modes).  Range reduction f = t - round(t) via the
            # fp32 magic constant: the fp32 ALU add rounds t to an integer.
            # sin(2*pi*t) = -sin(2*pi*f); the flip cancels between q and k.
            # m is the OUTER loop with all 8 score banks accumulating, so PE
            # consumes each frequency's features as soon as they exist.
            sc_all = ps_big.tile([128, NB, QH], mybir.dt.float32, tag="big",
                                 name="scall")
            for m in range(1, FM + 1):
                nu = float(m / (2.0 * FL))
                fqk = temps.tile([128, 3, 512], F16, tag="fqk", name="fqk")
                tq = temps.tile([128, QH], F16, tag="tq", name="tq")
                nc.vector.tensor_scalar(tq, qT2, nu, bq[:, 0:1], AOp.mult, AOp.add)
                rq = temps.tile([128, QH], F16, tag="rq", name="rq")
                nc.vector.tensor_scalar(rq, tq, MAGIC, MAGIC, AOp.add, AOp.subtract)
                nc.vector.tensor_tensor(fqk[:, 0, :], tq, rq, AOp.subtract)

                tk = temps.tile([128, KK], F16, tag="tk", name="tk")
                nc.vector.tensor_scalar(tk, kT2, nu, bk[:, 0:1], AOp.mult, AOp.add)
                rk = temps.tile([128, KK], F16, tag="rk", name="rk")
                nc.vector.tensor_scalar(rk, tk, MAGIC, MAGIC, AOp.add, AOp.subtract)
                kflat = fqk[:, 1:3, :].rearrange("p a b -> p (a b)")
                nc.vector.tensor_tensor(kflat, tk, rk, AOp.subtract)

                # one fused Sin over [q-features | k-features]
                sraw = temps.tile([128, 3, 512], F16, tag="sraw", name="sraw")
                nc.scalar.activation(sraw.rearrange("p a b -> p (a b)"),
                                     fqk.rearrange("p a b -> p (a b)"),
                                     Sin, scale=TWO_PI)
                qfm = temps.tile([128, QH], F16, tag="qfm", name="qfm")
                nc.vector.tensor_scalar(qfm, sraw[:, 0, :], wvc_sb[:, m - 1:m],
                                        None, AOp.mult)
                kfm = sraw[:, 1:3, :].rearrange("p a b -> p (a b)")
                for j in range(NB):
                    nc.tensor.matmul(sc_all[:, j, :], kfm[:, j * 128:(j + 1) * 128],
                                     qfm, start=(m == 1), stop=(m == FM))

            # ---------------- exp + attn @ [values | ones] ----------------
            # two wide exp calls (PSUM->SBUF); the valid-length mask is
            # pre-multiplied into vaug on the host (p*mask*v == (p)*(mask*v)),
            # so the exp outputs feed the attention matmuls directly.
            ehs = []
            for half in range(2):
                eh = temps.tile([128, NB // 2, QH], F16, tag="eh", name="eh")
                nc.scalar.activation(eh.rearrange("p a b -> p (a b)"),
                                     sc_all[:, half * 4:(half + 1) * 4, :]
                                     .rearrange("p a b -> p (a b)"), Exp)
                ehs.append(eh)

            av_all = ps_big.tile([128, QH // 128, 512], mybir.dt.float32,
                                 tag="big", name="avall")
            for qb in range(QH // 128):
                for j in range(NB):
                    pj = ehs[j // 4][:, j % 4, :]
                    nc.tensor.matmul(av_all[:, qb, :V + 1],
                                     pj[:, qb * 128:(qb + 1) * 128],
                                     v_aug[j], start=(j == 0), stop=(j == NB - 1))
            # all 4 denominators reciprocal'd in one DVE op, then per-qb
            # scale on the tail-idle ACT engine into one tile, single DMA out
            recip4 = outp.tile([128, QH // 128], mybir.dt.float32, tag="recip",
                               name="recip4")
            nc.vector.reciprocal(
                recip4, av_all[:, :, V:V + 1].rearrange("p a b -> p (a b)"))
            o_all = outp.tile([128, QH // 128, V], mybir.dt.float32, tag="osb",
                              name="o_all")
            for qb in range(QH // 128):
                nc.scalar.mul(o_all[:, qb, :], av_all[:, qb, 0:V], recip4[:, qb:qb + 1])
            nc.sync.dma_start(
                out_d.rearrange("(a p) v -> p a v", p=128), o_all)

    nc.finalize()
    return nc


def _build_runner(nc):
    """Cached multi-core PJRT runner (mirrors bass2jax.run_bass_via_pjrt's
    multi-core path, but keeps the jitted callable so repeat calls don't
    retrace/recompile)."""
    import jax
    import numpy as _np
    from jax.sharding import Mesh, PartitionSpec
    from jax.experimental.shard_map import shard_map
    from concourse import bass2jax, mybir

    bass2jax.install_neuronx_cc_hook()

    partition_name = nc.partition_id_tensor.name if nc.partition_id_tensor else None
    in_names, out_names, out_avals, zero_outs = [], [], [], []
    for alloc in nc.m.functions[0].allocations:
        if not isinstance(alloc, mybir.MemoryLocationSet):
            continue
        name = alloc.memorylocations[0].name
        if alloc.kind == "ExternalInput":
            if name != partition_name:
                in_names.append(name)
        elif alloc.kind == "ExternalOutput":
            shape = tuple(alloc.tensor_shape)
            dtype = mybir.dt.np(alloc.dtype)
            out_names.append(name)
            out_avals.append(jax.core.ShapedArray(shape, dtype))
            zero_outs.append(_np.zeros(shape, dtype))
    n_params = len(in_names)
    n_outs = len(out_avals)
    all_in_names = list(in_names) + list(out_names)
    if partition_name is not None:
        all_in_names.append(partition_name)
    donate = tuple(range(n_params, n_params + n_outs))

    def _body(*args):
        operands = list(args)
        if partition_name is not None:
            operands.append(bass2jax.partition_id_tensor())
        outs = bass2jax._bass_exec_p.bind(
            *operands,
            out_avals=tuple(out_avals),
            in_names=tuple(all_in_names),
            out_names=tuple(out_names),
            lowering_input_output_aliases=(),
            sim_require_finite=True,
            sim_require_nnan=True,
            nc=nc,
        )
        return tuple(outs)

    devices = jax.devices()[:NCORES]
    assert len(devices) == NCORES, f"need {NCORES} cores, have {len(jax.devices())}"
    mesh = Mesh(_np.asarray(devices), ("core",))
    in_specs = (PartitionSpec("core"),) * (n_params + n_outs)
    out_specs = (PartitionSpec("core"),) * n_outs
    sharded = jax.jit(
        shard_map(_body, mesh=mesh, in_specs=in_specs, out_specs=out_specs,
                  check_rep=False),
        donate_argnums=donate, keep_unused=True)

    def run(in_maps):
        per_core = [[_np.asarray(m[name]) for name in in_names] for m in in_maps]
        concat_in = [
            _np.concatenate([per_core[c][i] for c in range(NCORES)], axis=0)
            for i in range(n_params)
        ]
        concat_zeros = [
            _np.zeros((NCORES * z.shape[0], *z.shape[1:]), z.dtype) for z in zero_outs
        ]
        out_arrs = sharded(*concat_in, *concat_zeros)
        return [
            {
                name: _np.asarray(out_arrs[i]).reshape(NCORES, *out_avals[i].shape)[c]
                for i, name in enumerate(out_names)
            }
            for c in range(NCORES)
        ]

    return run


def get_nc(n_iters=1):
    key = f"nc{n_iters}"
    if key not in _STATE:
        _STATE[key] = _build_nc(n_iters)
    return _STATE[key]


def make_in_maps(queries, keys, values, valid_lens, W_q, W_k, w_v):
    queries = np.asarray(queries, dtype=np.float32)
    keys = np.asarray(keys, dtype=np.float32)
    values = np.asarray(values, dtype=np.float32)
    valid_lens = np.asarray(valid_lens)
    WqT = np.ascontiguousarray(np.asarray(W_q, dtype=np.float32).T.astype(np.float16))
    WkT = np.ascontiguousarray(np.asarray(W_k, dtype=np.float32).T.astype(np.float16))
    w_v = np.asarray(w_v, dtype=np.float32)
    wv2 = np.concatenate([w_v, w_v])
    wvc = np.ascontiguousarray(wv2[:, None] * COEFFS[None, :])
    in_maps = []
    for core in range(NCORES):
        b, hf = core // 2, core % 2
        L = int(valid_lens[b])
        mask = (np.arange(KK) < L).astype(np.float32)[:, None]
        vaug = (np.concatenate([values[b], np.ones((KK, 1), np.float32)],
                               axis=1) * mask).astype(np.float16)
        in_maps.append({
            "queriesT": np.ascontiguousarray(
                queries[b, hf * QH:(hf + 1) * QH, :].T.astype(np.float16)),
            "keysT": np.ascontiguousarray(keys[b].T.astype(np.float16)),
            "vaug": np.ascontiguousarray(vaug),
            "wqT": WqT,
            "wkT": WkT,
            "wvc": wvc,
        })
    return in_maps


def kernel(queries, keys, values, valid_lens, W_q, W_k, w_v):
    nc = get_nc()
    if "run" not in _STATE:
        _STATE["run"] = _build_runner(nc)
    in_maps = make_in_maps(queries, keys, values, valid_lens, W_q, W_k, w_v)
    results = _STATE["run"](in_maps)
    out = np.empty((B, QFULL, V), np.float32)
    for core in range(NCORES):
        b, hf = core // 2, core % 2
        out[b, hf * QH:(hf + 1) * QH, :] = results[core]["out"]
    return out



# revision 5
# speedup vs baseline: 1.2395x; 1.2395x over previous
"""Additive-attention fused kernel for one TRN2 chip (8 NeuronCores), v3.

Math (per batch b):
    q = queries @ W_q.T                       [Q, H]
    k = keys    @ W_k.T                       [K, H]
    scores[q,k] = sum_h w_v[h] * tanh(q[q,h] + k[k,h])
    attn = masked_softmax(scores, valid_len)  (mask: k >= L -> weight 0)
    out  = attn @ values                      [Q, V]

Sharding: data-parallel over batch B=4 x 2-way split of Q -> 8 cores,
each core handles [QH=512, :] of one batch.  No collectives.

Algorithm (v3): tanh(x+y) via an odd Fourier series (weighted LSQ fit,
frequencies m*pi/FL, m=1..FM):

    tanh(z) ~ sum_m c_m sin(pi m z / FL)
    sin(w(x+y)) = sin(wx)cos(wy) + cos(wx)sin(wy)  -> rank 2 per freq

v3 improvements over v2 (48 us measured):
  - K tiled to NB = ceil(max(valid_lens)/128) blocks (6 for the graded
    inputs, vs 8): scores/attn work on fully-masked k-blocks is skipped.
  - FM 10 -> 7 with a tighter FL=8.5 fit (measured end-to-end rel err
    5.3e-3 vs the 2e-2 gate; no projection clamp -- wraps are ~1e-9
    probability per element and bounded).
  - Per-m features by the Chebyshev-style recurrence
        S_m = 2cos(theta) * S_{m-1} - S_{m-2},   S_m = sin(m*theta + phi)
    (2 DVE tensor_tensor ops per m) instead of the v2 per-m chain
    DVE range-reduce -> ACT Sin -> DVE scale, which serialized ~4.6us/m
    across engines.  Only the m=1 sin/cos pair touches ACT Sin.
  - h-duplication done by the PE for free: host ships [W | W] so the
    projection lands as [128, n] in PSUM; ACT Sin reads PSUM directly
    with per-partition phase bias (no SBUF round-trip, no clamp pass).
  - c_m * w_v scaling on the otherwise-idle ACT engine (Copy with a
    per-partition scale AP) so DVE does only the 2 recurrence ops per m.
  - Dummy 1-col Sin/Exp activations issued early prefetch the two ACT
    tables (1.28us each) under DMA/matmul shadows.
  - fp16 output + fp16 staging halve the out/in DMA bytes.
"""

import math

import numpy as np

B, QFULL, KK = 4, 1024, 1024
D, H, V = 256, 64, 256
QH = 512            # Q rows per core
NCORES = 8

# tanh(z) ~ sum_m C[m-1] * sin(pi*m*z / FL)
FM = 7              # number of frequencies
FL = 8.5            # half-period

_STATE = {}


def _fit_coeffs():
    z = np.linspace(-FL, FL, 6001)
    w = np.exp(-z ** 2 / 8.0) + 1e-4
    A = np.sin(np.pi / FL * np.outer(z, np.arange(1, FM + 1)))
    c = np.linalg.lstsq(A * np.sqrt(w)[:, None], np.tanh(z) * np.sqrt(w),
                        rcond=None)[0]
    return c.astype(np.float32)


COEFFS = _fit_coeffs()


def _build_nc(n_iters=1, nb=6):
    import contextlib
    import concourse.tile as tile
    from concourse import bacc, mybir

    F32 = mybir.dt.float32
    F16 = mybir.dt.float16
    Sin = mybir.ActivationFunctionType.Sin
    Exp = mybir.ActivationFunctionType.Exp
    Copy = mybir.ActivationFunctionType.Copy
    AOp = mybir.AluOpType

    KKnb = nb * 128
    FW = QH + KKnb              # feature width: q cols | k cols
    NU = math.pi / FL           # sin(m * NU * x + phi)
    HPI = math.pi / 2.0
    # k-projection PSUM groups of <=512 f32 (one PSUM bank each)
    kgroups = []
    off = 0
    while off < KKnb:
        kgroups.append((off, min(512, KKnb - off)))
        off += 512

    nc = bacc.Bacc()
    qT_d = nc.declare_dram_parameter("queriesT", [D, QH], F16, isOutput=False)
    kT_d = nc.declare_dram_parameter("keysT", [D, KKnb], F16, isOutput=False)
    va_d = nc.declare_dram_parameter("vaug", [KKnb, V + 1], F16, isOutput=False)
    wqT_d = nc.declare_dram_parameter("wqTd", [D, 128], F16, isOutput=False)
    wkT_d = nc.declare_dram_parameter("wkTd", [D, 128], F16, isOutput=False)
    wvc_d = nc.declare_dram_parameter("wvc", [128, FM], F32, isOutput=False)
    out_d = nc.declare_dram_parameter("out", [QH, V], F16, isOutput=True)

    with tile.TileContext(nc) as tc:
        with (
            tc.tile_pool(name="singles", bufs=1) as singles,
            tc.tile_pool(name="sring", bufs=4) as sring,
            tc.tile_pool(name="temps", bufs=2) as temps,
            tc.tile_pool(name="qfmp", bufs=3) as qfmp,
            tc.tile_pool(name="outp", bufs=2) as outp,
            # one PSUM pool, tag-shared slot: projections -> score banks ->
            # attn@V accumulators reuse the space with Tile-inserted deps.
            tc.tile_pool(name="ps_big", bufs=1, space="PSUM") as ps_big,
            tc.For_i(0, n_iters, 1,
                     hint_engines=(mybir.EngineType.PE, mybir.EngineType.DVE,
                                   mybir.EngineType.Activation,
                                   mybir.EngineType.SP, mybir.EngineType.Pool),
                     staggered_reset=True)
            if n_iters > 1 else contextlib.nullcontext(),
        ):
            # -------- stage inputs (host pre-transposed / pre-masked) -------
            wq, wk = [], []
            for c in range(2):
                t = singles.tile([128, 128], F16, tag=f"wq{c}", name=f"wq{c}")
                nc.sync.dma_start(t, wqT_d[c * 128:(c + 1) * 128, :])
                wq.append(t)
                t2 = singles.tile([128, 128], F16, tag=f"wk{c}", name=f"wk{c}")
                nc.sync.dma_start(t2, wkT_d[c * 128:(c + 1) * 128, :])
                wk.append(t2)
            queriesT = []
            for c in range(2):
                t = singles.tile([128, QH], F16, tag=f"qTc{c}", name=f"qTc{c}")
                nc.sync.dma_start(t, qT_d[c * 128:(c + 1) * 128, :])
                queriesT.append(t)
            keysT = []
            for c in range(2):
                t = singles.tile([128, KKnb], F16, tag=f"kTc{c}", name=f"kTc{c}")
                nc.sync.dma_start(t, kT_d[c * 128:(c + 1) * 128, :])
                keysT.append(t)
            wvc_sb = singles.tile([128, FM], F32, tag="wvc", name="wvc")
            nc.sync.dma_start(wvc_sb, wvc_d[:, :])
            v_aug = []
            for i in range(nb):
                va = singles.tile([128, V + 1], F16, tag=f"vaug{i}", name=f"vaug{i}")
                nc.sync.dma_start(va, va_d[i * 128:(i + 1) * 128, :])
                v_aug.append(va)

            # per-partition phase biases (radians) and S0 columns
            bq = singles.tile([128, 1], F32, tag="bq", name="bq")
            nc.vector.memset(bq[0:H, :], 0.0)
            nc.vector.memset(bq[H:128, :], HPI)
            bk = singles.tile([128, 1], F32, tag="bk", name="bk")
            nc.vector.memset(bk[0:H, :], HPI)
            nc.vector.memset(bk[H:128, :], 0.0)
            bh = singles.tile([128, 1], F32, tag="bh", name="bh")
            nc.vector.memset(bh, HPI)
            s0q = singles.tile([128, 1], F32, tag="s0q", name="s0q")
            nc.vector.memset(s0q[0:H, :], 0.0)
            nc.vector.memset(s0q[H:128, :], 1.0)
            s0k = singles.tile([128, 1], F32, tag="s0k", name="s0k")
            nc.vector.memset(s0k[0:H, :], 1.0)
            nc.vector.memset(s0k[H:128, :], 0.0)

            # dummy 1-col Sin: prefetches the trig ACT table during DMA/proj
            dumb = singles.tile([128, 1], F32, tag="dumb", name="dumb")
            nc.vector.memset(dumb, 0.0)
            dsin = singles.tile([128, 1], F32, tag="dsin", name="dsin")
            nc.scalar.activation(dsin, dumb, Sin)

            # ------------- projections -> PSUM [128, 1+len(kgroups) banks] --
            # lhsT = [W | W] (host-duplicated columns) -> 128 out partitions:
            # rows 0:64 and 64:128 both hold the h-projection, so the two
            # phase halves (sin/cos) come for free.
            ng = 1 + len(kgroups)
            psp = ps_big.tile([128, ng, 512], F32, tag="big", name="psp")
            for c in range(2):
                nc.tensor.matmul(psp[:, 0, 0:QH], wq[c], queriesT[c],
                                 start=(c == 0), stop=(c == 1))
            for gi, (goff, glen) in enumerate(kgroups):
                for c in range(2):
                    nc.tensor.matmul(psp[:, 1 + gi, 0:glen], wk[c],
                                     keysT[c][:, goff:goff + glen],
                                     start=(c == 0), stop=(c == 1))

            # ---------------- S1 / C via ACT Sin (PSUM -> SBUF fp16) --------
            # S1 = sin(NU*x + phi(p));  C = 2*cos(NU*x) via sin(.+pi/2).
            s1 = sring.tile([128, FW], F16, tag="S", name="s1")
            nc.scalar.activation(s1[:, 0:QH], psp[:, 0, 0:QH], Sin,
                                 scale=NU, bias=bq[:, 0:1])
            for gi, (goff, glen) in enumerate(kgroups):
                nc.scalar.activation(s1[:, QH + goff:QH + goff + glen],
                                     psp[:, 1 + gi, 0:glen], Sin,
                                     scale=NU, bias=bk[:, 0:1])
            craw = temps.tile([128, FW], F16, tag="craw", name="craw")
            nc.scalar.activation(craw[:, 0:QH], psp[:, 0, 0:QH], Sin,
                                 scale=NU, bias=bh[:, 0:1])
            for gi, (goff, glen) in enumerate(kgroups):
                nc.scalar.activation(craw[:, QH + goff:QH + goff + glen],
                                     psp[:, 1 + gi, 0:glen], Sin,
                                     scale=NU, bias=bh[:, 0:1])
            # dummy 1-col Exp right after the last Sin: swaps to the exp
            # table during the m-loop (ACT Copy runs from any table).
            dexp = singles.tile([128, 1], F32, tag="dexp", name="dexp")
            nc.scalar.activation(dexp, dumb, Exp)

            cc = singles.tile([128, FW], F16, tag="cc", name="cc")
            nc.vector.tensor_scalar(cc, craw, 2.0, None, AOp.mult)

            # ------- m-loop: recurrence + c_m*w_v scale + PE accumulation ---
            sc_all = ps_big.tile([128, nb, QH], F32, tag="big", name="scall")
            s_prev2, s_prev1 = None, s1
            for m in range(1, FM + 1):
                if m == 2:
                    p = temps.tile([128, FW], F16, tag="P", name=f"p{m}")
                    nc.vector.tensor_tensor(p, cc, s_prev1, AOp.mult)
                    sm = sring.tile([128, FW], F16, tag="S", name=f"s{m}")
                    nc.vector.tensor_scalar(sm[:, 0:QH], p[:, 0:QH],
                                            s0q[:, 0:1], None, AOp.subtract)
                    nc.vector.tensor_scalar(sm[:, QH:FW], p[:, QH:FW],
                                            s0k[:, 0:1], None, AOp.subtract)
                    s_prev2, s_prev1 = s_prev1, sm
                elif m > 2:
                    p = temps.tile([128, FW], F16, tag="P", name=f"p{m}")
                    nc.vector.tensor_tensor(p, cc, s_prev1, AOp.mult)
                    sm = sring.tile([128, FW], F16, tag="S", name=f"s{m}")
                    nc.vector.tensor_tensor(sm, p, s_prev2, AOp.subtract)
                    s_prev2, s_prev1 = s_prev1, sm
                sm = s_prev1
                qfm = qfmp.tile([128, QH], F16, tag="qfm", name=f"qfm{m}")
                nc.scalar.activation(qfm, sm[:, 0:QH], Copy,
                                     scale=wvc_sb[:, m - 1:m])
                for j in range(nb):
                    nc.tensor.matmul(sc_all[:, j, :],
                                     sm[:, QH + j * 128:QH + (j + 1) * 128],
                                     qfm, start=(m == 1), stop=(m == FM))

            # ---------------- exp + attn @ [values | ones] ------------------
            nh0 = (nb + 1) // 2
            ehs = []
            for half, (hoff, hlen) in enumerate([(0, nh0), (nh0, nb - nh0)]):
                if hlen == 0:
                    continue
                eh = temps.tile([128, hlen, QH], F16, tag=f"eh{half}",
                                name=f"eh{half}")
                nc.scalar.activation(eh.rearrange("p a b -> p (a b)"),
                                     sc_all[:, hoff:hoff + hlen, :]
                                     .rearrange("p a b -> p (a b)"), Exp)
                ehs.append((hoff, hlen, eh))

            nqb = QH // 128
            av_all = ps_big.tile([128, nqb, 512], F32, tag="big", name="avall")
            for qb in range(nqb):
                for j in range(nb):
                    for (hoff, hlen, eh) in ehs:
                        if hoff <= j < hoff + hlen:
                            pj = eh[:, j - hoff, :]
                    nc.tensor.matmul(av_all[:, qb, :V + 1],
                                     pj[:, qb * 128:(qb + 1) * 128],
                                     v_aug[j], start=(j == 0), stop=(j == nb - 1))
            recip = outp.tile([128, nqb], F32, tag="recip", name="recip")
            nc.vector.reciprocal(
                recip, av_all[:, :, V:V + 1].rearrange("p a b -> p (a b)"))
            o_all = outp.tile([128, nqb, V], F16, tag="osb", name="o_all")
            for qb in range(nqb):
                nc.scalar.mul(o_all[:, qb, :], av_all[:, qb, 0:V],
                              recip[:, qb:qb + 1])
            nc.sync.dma_start(
                out_d.rearrange("(a p) v -> p a v", p=128), o_all)

    nc.finalize()
    return nc


def _build_runner(nc):
    """Cached multi-core PJRT runner (mirrors bass2jax.run_bass_via_pjrt's
    multi-core path, but keeps the jitted callable so repeat calls don't
    retrace/recompile)."""
    import jax
    import numpy as _np
    from jax.sharding import Mesh, PartitionSpec
    from jax.experimental.shard_map import shard_map
    from concourse import bass2jax, mybir

    bass2jax.install_neuronx_cc_hook()

    partition_name = nc.partition_id_tensor.name if nc.partition_id_tensor else None
    in_names, out_names, out_avals, zero_outs = [], [], [], []
    for alloc in nc.m.functions[0].allocations:
        if not isinstance(alloc, mybir.MemoryLocationSet):
            continue
        name = alloc.memorylocations[0].name
        if alloc.kind == "ExternalInput":
            if name != partition_name:
                in_names.append(name)
        elif alloc.kind == "ExternalOutput":
            shape = tuple(alloc.tensor_shape)
            dtype = mybir.dt.np(alloc.dtype)
            out_names.append(name)
            out_avals.append(jax.core.ShapedArray(shape, dtype))
            zero_outs.append(_np.zeros(shape, dtype))
    n_params = len(in_names)
    n_outs = len(out_avals)
    all_in_names = list(in_names) + list(out_names)
    if partition_name is not None:
        all_in_names.append(partition_name)
    donate = tuple(range(n_params, n_params + n_outs))

    def _body(*args):
        operands = list(args)
        if partition_name is not None:
            operands.append(bass2jax.partition_id_tensor())
        outs = bass2jax._bass_exec_p.bind(
            *operands,
            out_avals=tuple(out_avals),
            in_names=tuple(all_in_names),
            out_names=tuple(out_names),
            lowering_input_output_aliases=(),
            sim_require_finite=True,
            sim_require_nnan=True,
            nc=nc,
        )
        return tuple(outs)

    devices = jax.devices()[:NCORES]
    assert len(devices) == NCORES, f"need {NCORES} cores, have {len(jax.devices())}"
    mesh = Mesh(_np.asarray(devices), ("core",))
    in_specs = (PartitionSpec("core"),) * (n_params + n_outs)
    out_specs = (PartitionSpec("core"),) * n_outs
    sharded = jax.jit(
        shard_map(_body, mesh=mesh, in_specs=in_specs, out_specs=out_specs,
                  check_rep=False),
        donate_argnums=donate, keep_unused=True)

    def run(in_maps):
        per_core = [[_np.asarray(m[name]) for name in in_names] for m in in_maps]
        concat_in = [
            _np.concatenate([per_core[c][i] for c in range(NCORES)], axis=0)
            for i in range(n_params)
        ]
        concat_zeros = [
            _np.zeros((NCORES * z.shape[0], *z.shape[1:]), z.dtype) for z in zero_outs
        ]
        out_arrs = sharded(*concat_in, *concat_zeros)
        return [
            {
                name: _np.asarray(out_arrs[i]).reshape(NCORES, *out_avals[i].shape)[c]
                for i, name in enumerate(out_names)
            }
            for c in range(NCORES)
        ]

    return run


def get_nc(n_iters=1, nb=6):
    key = f"nc{n_iters}_{nb}"
    if key not in _STATE:
        _STATE[key] = _build_nc(n_iters, nb)
    return _STATE[key]


def make_in_maps(queries, keys, values, valid_lens, W_q, W_k, w_v):
    queries = np.asarray(queries, dtype=np.float32)
    keys = np.asarray(keys, dtype=np.float32)
    values = np.asarray(values, dtype=np.float32)
    valid_lens = np.asarray(valid_lens)
    nb = max(1, min(KK, int(valid_lens.max()) + 127) // 128)
    KKnb = nb * 128
    WqT = np.asarray(W_q, dtype=np.float32).T.astype(np.float16)
    WkT = np.asarray(W_k, dtype=np.float32).T.astype(np.float16)
    WqTd = np.ascontiguousarray(np.concatenate([WqT, WqT], axis=1))
    WkTd = np.ascontiguousarray(np.concatenate([WkT, WkT], axis=1))
    w_v = np.asarray(w_v, dtype=np.float32)
    wv2 = np.concatenate([w_v, w_v])
    wvc = np.ascontiguousarray(wv2[:, None] * COEFFS[None, :])
    in_maps = []
    for core in range(NCORES):
        b, hf = core // 2, core % 2
        L = int(valid_lens[b])
        mask = (np.arange(KKnb) < L).astype(np.float32)[:, None]
        vaug = (np.concatenate([values[b, :KKnb], np.ones((KKnb, 1), np.float32)],
                               axis=1) * mask).astype(np.float16)
        in_maps.append({
            "queriesT": np.ascontiguousarray(
                queries[b, hf * QH:(hf + 1) * QH, :].T.astype(np.float16)),
            "keysT": np.ascontiguousarray(keys[b, :KKnb].T.astype(np.float16)),
            "vaug": np.ascontiguousarray(vaug),
            "wqTd": WqTd,
            "wkTd": WkTd,
            "wvc": wvc,
        })
    return in_maps


def kernel(queries, keys, values, valid_lens, W_q, W_k, w_v):
    in_maps = make_in_maps(queries, keys, values, valid_lens, W_q, W_k, w_v)
    nb = in_maps[0]["keysT"].shape[1] // 128
    nc = get_nc(1, nb)
    rkey = f"run1_{nb}"
    if rkey not in _STATE:
        _STATE[rkey] = _build_runner(nc)
    results = _STATE[rkey](in_maps)
    out = np.empty((B, QFULL, V), np.float32)
    for core in range(NCORES):
        b, hf = core // 2, core % 2
        out[b, hf * QH:(hf + 1) * QH, :] = results[core]["out"].astype(np.float32)
    return out


# revision 17
# speedup vs baseline: 1.3171x; 1.0626x over previous
"""Additive-attention fused kernel for one TRN2 chip (8 NeuronCores), v3.

Math (per batch b):
    q = queries @ W_q.T                       [Q, H]
    k = keys    @ W_k.T                       [K, H]
    scores[q,k] = sum_h w_v[h] * tanh(q[q,h] + k[k,h])
    attn = masked_softmax(scores, valid_len)  (mask: k >= L -> weight 0)
    out  = attn @ values                      [Q, V]

Sharding: data-parallel over batch B=4 x 2-way split of Q -> 8 cores,
each core handles [QH=512, :] of one batch.  No collectives.

Algorithm (v3): tanh(x+y) via an odd Fourier series (weighted LSQ fit,
frequencies m*pi/FL, m=1..FM):

    tanh(z) ~ sum_m c_m sin(pi m z / FL)
    sin(w(x+y)) = sin(wx)cos(wy) + cos(wx)sin(wy)  -> rank 2 per freq

v3 improvements over v2 (48 us measured):
  - K tiled to NB = ceil(max(valid_lens)/128) blocks (6 for the graded
    inputs, vs 8): scores/attn work on fully-masked k-blocks is skipped.
  - FM 10 -> 7 with a tighter FL=8.5 fit (measured end-to-end rel err
    5.3e-3 vs the 2e-2 gate; no projection clamp -- wraps are ~1e-9
    probability per element and bounded).
  - Per-m features by the Chebyshev-style recurrence
        S_m = 2cos(theta) * S_{m-1} - S_{m-2},   S_m = sin(m*theta + phi)
    (2 DVE tensor_tensor ops per m) instead of the v2 per-m chain
    DVE range-reduce -> ACT Sin -> DVE scale, which serialized ~4.6us/m
    across engines.  Only the m=1 sin/cos pair touches ACT Sin.
  - h-duplication done by the PE for free: host ships [W | W] so the
    projection lands as [128, n] in PSUM; ACT Sin reads PSUM directly
    with per-partition phase bias (no SBUF round-trip, no clamp pass).
  - c_m * w_v scaling on the otherwise-idle ACT engine (Copy with a
    per-partition scale AP) so DVE does only the 2 recurrence ops per m.
  - Dummy 1-col Sin/Exp activations issued early prefetch the two ACT
    tables (1.28us each) under DMA/matmul shadows.
  - fp16 output + fp16 staging halve the out/in DMA bytes.
"""

import math

import numpy as np

B, QFULL, KK = 4, 1024, 1024
D, H, V = 256, 64, 256
QH = 512            # Q rows per core
NCORES = 8

# tanh(z) ~ sum_m C[m-1] * sin(pi*m*z / FL)
FM = 7              # number of frequencies
FL = 8.5            # half-period

_STATE = {}


def _fit_coeffs():
    z = np.linspace(-FL, FL, 6001)
    w = np.exp(-z ** 2 / 8.0) + 1e-4
    A = np.sin(np.pi / FL * np.outer(z, np.arange(1, FM + 1)))
    c = np.linalg.lstsq(A * np.sqrt(w)[:, None], np.tanh(z) * np.sqrt(w),
                        rcond=None)[0]
    return c.astype(np.float32)


COEFFS = _fit_coeffs()


def _build_nc(n_iters=1, nb=6, fm=None, skip_mloop=False, skip_tail=False,
              skip_setup=False, skip_proj=False):
    import contextlib
    import concourse.tile as tile
    from concourse import bacc, mybir

    F32 = mybir.dt.float32
    F16 = mybir.dt.float16
    Sin = mybir.ActivationFunctionType.Sin
    Exp = mybir.ActivationFunctionType.Exp
    Copy = mybir.ActivationFunctionType.Copy
    AOp = mybir.AluOpType

    fm = fm if fm is not None else FM
    KKnb = nb * 128
    FW = QH + KKnb              # feature width: q cols | k cols
    NU = math.pi / FL           # sin(m * NU * x + phi)
    HPI = math.pi / 2.0
    # k-projection PSUM groups of <=512 f32 (one PSUM bank each)
    kgroups = []
    off = 0
    while off < KKnb:
        kgroups.append((off, min(512, KKnb - off)))
        off += 512

    nc = bacc.Bacc()
    qT_d = nc.declare_dram_parameter("queriesT", [D, QH], F16, isOutput=False)
    kT_d = nc.declare_dram_parameter("keysT", [D, KKnb], F16, isOutput=False)
    va_d = nc.declare_dram_parameter("vaug", [KKnb, V + 1], F16, isOutput=False)
    wqk_d = nc.declare_dram_parameter("wqkd", [D, 256], F16, isOutput=False)
    wvc_d = nc.declare_dram_parameter("wvc", [128, FM], F32, isOutput=False)
    out_d = nc.declare_dram_parameter("out", [QH, V], F16, isOutput=True)

    with tile.TileContext(nc) as tc:
        with (
            tc.tile_pool(name="singles", bufs=1) as singles,
            tc.tile_pool(name="sring", bufs=4) as sring,
            tc.tile_pool(name="temps", bufs=2) as temps,
            tc.tile_pool(name="qfmp", bufs=3) as qfmp,
            tc.tile_pool(name="outp", bufs=2) as outp,
            # one PSUM pool, tag-shared slot: projections -> score banks ->
            # attn@V accumulators reuse the space with Tile-inserted deps.
            tc.tile_pool(name="ps_big", bufs=1, space="PSUM") as ps_big,
            tc.For_i(0, n_iters, 1,
                     hint_engines=(mybir.EngineType.PE, mybir.EngineType.DVE,
                                   mybir.EngineType.Activation,
                                   mybir.EngineType.SP, mybir.EngineType.Pool),
                     staggered_reset=True)
            if n_iters > 1 else contextlib.nullcontext(),
        ):
            # -------- stage inputs (host pre-transposed / pre-masked) -------
            # One consolidated DMA per logical input (each dma_start costs
            # ~0.6us generation + 0.9us sem propagation), spread over the
            # SP and DVE hardware DGE queues so generation overlaps.  The
            # output DMA lives on the ACT queue, so in the For_i timing
            # loop the next iteration's input staging is not queued behind
            # this iteration's output.
            wqk_t = singles.tile([128, 2, 256], F16, tag="wqk", name="wqk")
            nc.sync.dma_start(
                wqk_t, wqk_d.rearrange("(c p) w -> p c w", p=128))
            wq = [wqk_t[:, c, 0:128] for c in range(2)]
            wk = [wqk_t[:, c, 128:256] for c in range(2)]
            qt_t = singles.tile([128, 2, QH], F16, tag="qT", name="qT")
            nc.sync.dma_start(
                qt_t, qT_d.rearrange("(c p) q -> p c q", p=128))
            queriesT = [qt_t[:, c, :] for c in range(2)]
            kt_t = singles.tile([128, 2, KKnb], F16, tag="kT", name="kT")
            nc.sync.dma_start(
                kt_t, kT_d.rearrange("(c p) k -> p c k", p=128))
            keysT = [kt_t[:, c, :] for c in range(2)]
            wvc_sb = singles.tile([128, FM], F32, tag="wvc", name="wvc")
            nc.sync.dma_start(wvc_sb, wvc_d[:, :])
            va_t = singles.tile([128, nb, V + 1], F16, tag="vaug", name="vaug")
            nc.sync.dma_start(
                va_t, va_d.rearrange("(a p) v -> p a v", p=128))
            v_aug = [va_t[:, i, :] for i in range(nb)]

            # per-partition phase biases (radians) and S0 columns
            bq = singles.tile([128, 1], F32, tag="bq", name="bq")
            nc.vector.memset(bq[0:H, :], 0.0)
            nc.vector.memset(bq[H:128, :], HPI)
            bk = singles.tile([128, 1], F32, tag="bk", name="bk")
            nc.vector.memset(bk[0:H, :], HPI)
            nc.vector.memset(bk[H:128, :], 0.0)
            bh = singles.tile([128, 1], F32, tag="bh", name="bh")
            nc.vector.memset(bh, HPI)
            s0q = singles.tile([128, 1], F32, tag="s0q", name="s0q")
            nc.vector.memset(s0q[0:H, :], 0.0)
            nc.vector.memset(s0q[H:128, :], 1.0)
            s0k = singles.tile([128, 1], F32, tag="s0k", name="s0k")
            nc.vector.memset(s0k[0:H, :], 1.0)
            nc.vector.memset(s0k[H:128, :], 0.0)

            # dummy 1-col Sin: prefetches the trig ACT table during DMA/proj
            dumb = singles.tile([128, 1], F32, tag="dumb", name="dumb")
            nc.vector.memset(dumb, 0.0)
            dsin = singles.tile([128, 1], F32, tag="dsin", name="dsin")
            nc.scalar.activation(dsin, dumb, Sin)

            # ------------- projections -> PSUM [128, 1+len(kgroups) banks] --
            # lhsT = [W | W] (host-duplicated columns) -> 128 out partitions:
            # rows 0:64 and 64:128 both hold the h-projection, so the two
            # phase halves (sin/cos) come for free.
            ng = 1 + len(kgroups)
            psp = ps_big.tile([128, ng, 512], F32, tag="big", name="psp")
            for c in range(2 if not skip_proj else 0):
                nc.tensor.matmul(psp[:, 0, 0:QH], wq[c], queriesT[c],
                                 start=(c == 0), stop=(c == 1))
            for gi, (goff, glen) in enumerate(kgroups if not skip_proj else []):
                for c in range(2):
                    nc.tensor.matmul(psp[:, 1 + gi, 0:glen], wk[c],
                                     keysT[c][:, goff:goff + glen],
                                     start=(c == 0), stop=(c == 1))

            # ---------------- S1 / C via ACT Sin (PSUM -> SBUF fp16) --------
            # S1 = sin(NU*x + phi(p));  C = 2*cos(NU*x) via sin(.+pi/2).
            s1 = sring.tile([128, FW], F16, tag="S", name="s1")
            if skip_setup:
                kgroups_s = []
            else:
                kgroups_s = kgroups
                nc.scalar.activation(s1[:, 0:QH], psp[:, 0, 0:QH], Sin,
                                     scale=NU, bias=bq[:, 0:1])
            for gi, (goff, glen) in enumerate(kgroups_s):
                nc.scalar.activation(s1[:, QH + goff:QH + goff + glen],
                                     psp[:, 1 + gi, 0:glen], Sin,
                                     scale=NU, bias=bk[:, 0:1])
            craw = temps.tile([128, FW], F16, tag="craw", name="craw")
            if not skip_setup:
                nc.scalar.activation(craw[:, 0:QH], psp[:, 0, 0:QH], Sin,
                                     scale=NU, bias=bh[:, 0:1])
            for gi, (goff, glen) in enumerate(kgroups_s):
                nc.scalar.activation(craw[:, QH + goff:QH + goff + glen],
                                     psp[:, 1 + gi, 0:glen], Sin,
                                     scale=NU, bias=bh[:, 0:1])
            # dummy 1-col Exp right after the last Sin: swaps to the exp
            # table during the m-loop (ACT Copy runs from any table).
            dexp = singles.tile([128, 1], F32, tag="dexp", name="dexp")
            nc.scalar.activation(dexp, dumb, Exp)

            cc = singles.tile([128, FW], F16, tag="cc", name="cc")
            if not skip_setup:
                nc.vector.tensor_scalar(cc, craw, 2.0, None, AOp.mult)

            # ------- m-loop: recurrence + c_m*w_v scale + PE accumulation ---
            sc_all = ps_big.tile([128, nb, QH], F32, tag="big", name="scall")
            s_prev2, s_prev1 = None, s1
            for m in range(1, (0 if skip_mloop else fm) + 1):
                if m == 2:
                    p = temps.tile([128, FW], F16, tag="P", name=f"p{m}")
                    nc.vector.tensor_tensor(p, cc, s_prev1, AOp.mult)
                    sm = sring.tile([128, FW], F16, tag="S", name=f"s{m}")
                    nc.vector.tensor_scalar(sm[:, 0:QH], p[:, 0:QH],
                                            s0q[:, 0:1], None, AOp.subtract)
                    nc.vector.tensor_scalar(sm[:, QH:FW], p[:, QH:FW],
                                            s0k[:, 0:1], None, AOp.subtract)
                    s_prev2, s_prev1 = s_prev1, sm
                elif m > 2:
                    p = temps.tile([128, FW], F16, tag="P", name=f"p{m}")
                    nc.vector.tensor_tensor(p, cc, s_prev1, AOp.mult)
                    sm = sring.tile([128, FW], F16, tag="S", name=f"s{m}")
                    nc.vector.tensor_tensor(sm, p, s_prev2, AOp.subtract)
                    s_prev2, s_prev1 = s_prev1, sm
                sm = s_prev1
                qfm = qfmp.tile([128, QH], F16, tag="qfm", name=f"qfm{m}")
                nc.scalar.activation(qfm, sm[:, 0:QH], Copy,
                                     scale=wvc_sb[:, m - 1:m])
                for j in range(nb):
                    nc.tensor.matmul(sc_all[:, j, :],
                                     sm[:, QH + j * 128:QH + (j + 1) * 128],
                                     qfm, start=(m == 1), stop=(m == fm))

            # ---------------- exp + attn @ [values | ones] ------------------
            if skip_mloop or skip_tail:
                src = s_prev1 if skip_tail and not skip_mloop else (
                    queriesT[0] if skip_setup else craw)
                o_all = outp.tile([128, QH // 128, V], F16, tag="osb", name="o_all")
                for qb in range(QH // 128):
                    so = (qb * V) % (src.shape[-1] - V + 1)
                    nc.scalar.activation(o_all[:, qb, :],
                                         src[:, so:so + V], Copy)
                nc.sync.dma_start(
                    out_d.rearrange("(a p) v -> p a v", p=128), o_all)
            else:
                # exp per half-bank-group; attn@V in matching waves so the
                # second exp overlaps the first wave's matmuls.
                nqb = QH // 128
                nh0 = (nb + 1) // 2
                av_all = ps_big.tile([128, nqb, 512], F32, tag="big", name="avall")
                for half, (hoff, hlen) in enumerate([(0, nh0), (nh0, nb - nh0)]):
                    if hlen == 0:
                        continue
                    eh = temps.tile([128, hlen, QH], F16, tag=f"eh{half}",
                                    name=f"eh{half}")
                    nc.scalar.activation(eh.rearrange("p a b -> p (a b)"),
                                         sc_all[:, hoff:hoff + hlen, :]
                                         .rearrange("p a b -> p (a b)"), Exp)
                    for qb in range(nqb):
                        for j in range(hoff, hoff + hlen):
                            nc.tensor.matmul(av_all[:, qb, :V + 1],
                                             eh[:, j - hoff,
                                                qb * 128:(qb + 1) * 128],
                                             v_aug[j], start=(j == 0),
                                             stop=(j == nb - 1))
                o_all = outp.tile([128, nqb, V], F16, tag="osb", name="o_all")
                for qb in range(nqb):
                    recip = outp.tile([128, 1], F32, tag=f"recip{qb}",
                                      name=f"recip{qb}")
                    nc.vector.reciprocal(recip, av_all[:, qb, V:V + 1])
                    nc.scalar.mul(o_all[:, qb, :], av_all[:, qb, 0:V],
                                  recip[:, 0:1])
                nc.scalar.dma_start(
                    out_d.rearrange("(a p) v -> p a v", p=128), o_all)

    nc.finalize()
    return nc


def _build_runner(nc):
    """Cached multi-core PJRT runner (mirrors bass2jax.run_bass_via_pjrt's
    multi-core path, but keeps the jitted callable so repeat calls don't
    retrace/recompile)."""
    import jax
    import numpy as _np
    from jax.sharding import Mesh, PartitionSpec
    from jax.experimental.shard_map import shard_map
    from concourse import bass2jax, mybir

    bass2jax.install_neuronx_cc_hook()

    partition_name = nc.partition_id_tensor.name if nc.partition_id_tensor else None
    in_names, out_names, out_avals, zero_outs = [], [], [], []
    for alloc in nc.m.functions[0].allocations:
        if not isinstance(alloc, mybir.MemoryLocationSet):
            continue
        name = alloc.memorylocations[0].name
        if alloc.kind == "ExternalInput":
            if name != partition_name:
                in_names.append(name)
        elif alloc.kind == "ExternalOutput":
            shape = tuple(alloc.tensor_shape)
            dtype = mybir.dt.np(alloc.dtype)
            out_names.append(name)
            out_avals.append(jax.core.ShapedArray(shape, dtype))
            zero_outs.append(_np.zeros(shape, dtype))
    n_params = len(in_names)
    n_outs = len(out_avals)
    all_in_names = list(in_names) + list(out_names)
    if partition_name is not None:
        all_in_names.append(partition_name)
    donate = tuple(range(n_params, n_params + n_outs))

    def _body(*args):
        operands = list(args)
        if partition_name is not None:
            operands.append(bass2jax.partition_id_tensor())
        outs = bass2jax._bass_exec_p.bind(
            *operands,
            out_avals=tuple(out_avals),
            in_names=tuple(all_in_names),
            out_names=tuple(out_names),
            lowering_input_output_aliases=(),
            sim_require_finite=True,
            sim_require_nnan=True,
            nc=nc,
        )
        return tuple(outs)

    devices = jax.devices()[:NCORES]
    assert len(devices) == NCORES, f"need {NCORES} cores, have {len(jax.devices())}"
    mesh = Mesh(_np.asarray(devices), ("core",))
    in_specs = (PartitionSpec("core"),) * (n_params + n_outs)
    out_specs = (PartitionSpec("core"),) * n_outs
    sharded = jax.jit(
        shard_map(_body, mesh=mesh, in_specs=in_specs, out_specs=out_specs,
                  check_rep=False),
        donate_argnums=donate, keep_unused=True)

    def run(in_maps):
        per_core = [[_np.asarray(m[name]) for name in in_names] for m in in_maps]
        concat_in = [
            _np.concatenate([per_core[c][i] for c in range(NCORES)], axis=0)
            for i in range(n_params)
        ]
        concat_zeros = [
            _np.zeros((NCORES * z.shape[0], *z.shape[1:]), z.dtype) for z in zero_outs
        ]
        out_arrs = sharded(*concat_in, *concat_zeros)
        return [
            {
                name: _np.asarray(out_arrs[i]).reshape(NCORES, *out_avals[i].shape)[c]
                for i, name in enumerate(out_names)
            }
            for c in range(NCORES)
        ]

    return run


def get_nc(n_iters=1, nb=6):
    key = f"nc{n_iters}_{nb}"
    if key not in _STATE:
        _STATE[key] = _build_nc(n_iters, nb)
    return _STATE[key]


def make_in_maps(queries, keys, values, valid_lens, W_q, W_k, w_v):
    queries = np.asarray(queries, dtype=np.float32)
    keys = np.asarray(keys, dtype=np.float32)
    values = np.asarray(values, dtype=np.float32)
    valid_lens = np.asarray(valid_lens)
    nb = max(1, min(KK, int(valid_lens.max()) + 127) // 128)
    KKnb = nb * 128
    WqT = np.asarray(W_q, dtype=np.float32).T.astype(np.float16)
    WkT = np.asarray(W_k, dtype=np.float32).T.astype(np.float16)
    Wqk = np.ascontiguousarray(np.concatenate([WqT, WqT, WkT, WkT], axis=1))
    w_v = np.asarray(w_v, dtype=np.float32)
    wv2 = np.concatenate([w_v, w_v])
    wvc = np.ascontiguousarray(wv2[:, None] * COEFFS[None, :])
    in_maps = []
    for core in range(NCORES):
        b, hf = core // 2, core % 2
        L = int(valid_lens[b])
        mask = (np.arange(KKnb) < L).astype(np.float32)[:, None]
        vaug = (np.concatenate([values[b, :KKnb], np.ones((KKnb, 1), np.float32)],
                               axis=1) * mask).astype(np.float16)
        in_maps.append({
            "queriesT": np.ascontiguousarray(
                queries[b, hf * QH:(hf + 1) * QH, :].T.astype(np.float16)),
            "keysT": np.ascontiguousarray(keys[b, :KKnb].T.astype(np.float16)),
            "vaug": np.ascontiguousarray(vaug),
            "wqkd": Wqk,
            "wvc": wvc,
        })
    return in_maps


def kernel(queries, keys, values, valid_lens, W_q, W_k, w_v):
    in_maps = make_in_maps(queries, keys, values, valid_lens, W_q, W_k, w_v)
    nb = in_maps[0]["keysT"].shape[1] // 128
    nc = get_nc(1, nb)
    rkey = f"run1_{nb}"
    if rkey not in _STATE:
        _STATE[rkey] = _build_runner(nc)
    results = _STATE[rkey](in_maps)
    out = np.empty((B, QFULL, V), np.float32)
    for core in range(NCORES):
        b, hf = core // 2, core % 2
        out[b, hf * QH:(hf + 1) * QH, :] = results[core]["out"].astype(np.float32)
    return out
